# revision 4
# baseline (speedup 1.0000x reference)
"""Trainium2 Bass kernel for nn_ExpandEvecs.

Computes, for evecs [B=4, C=1, N=1024, K=16]:
    outers[b,k,i,j] = evecs[b,0,i,k] * evecs[b,0,j,k]
    cube = cumsum(outers, axis=k)  ->  [B, K, N, N]
i.e. cube[b,l] = V[:, :l+1] @ V[:, :l+1]^T  (Gram expansion per level).

Sharding: 8 cores = 4 batches x 2 row-halves. Core c (b=c//2, h=c%2)
computes all 16 levels for its 512-row half of batch b:
    out_c[l] = V[h*512:(h+1)*512, :l+1] @ V[:, :l+1]^T     [16, 512, 1024]
No inter-core communication. The 256 MiB f32 output (32 MiB/core) makes
this an HBM-write-bound problem (~94 us/core roofline at ~358 GB/s).

Per-core kernel: V^T is loaded to SBUF once ([16,1024], 64 KiB); each
(level, 128-row block) is two fp32r matmuls on TensorE into one 2-bank
PSUM tile, a PSUM->SBUF copy (alternating Vector/Scalar engines), and a
contiguous 512 KiB DMA store.
"""

import numpy as np

import concourse.mybir as mybir
from concourse import bacc, bass
from concourse.tile import TileContext
from concourse.bass_utils import run_bass_kernel_spmd

B, C, N, K = 4, 1, 1024, 16
NCORES = 8
HALF = N // 2          # rows per core
RB = HALF // 128       # 128-row blocks per core (4)

F32 = mybir.dt.float32
F32R = mybir.dt.float32r

_nc_cache = None


def _build():
    nc = bacc.Bacc(None, target_bir_lowering=False)
    vt_d = nc.declare_dram_parameter("vt", [K, N], F32R, isOutput=False)
    vtl_d = nc.declare_dram_parameter("vtl", [K, HALF], F32R, isOutput=False)
    out_d = nc.declare_dram_parameter("out", [K, HALF, N], F32, isOutput=True)

    with TileContext(nc) as tc:
        with (
            tc.tile_pool(name="vpool", bufs=1) as vpool,
            tc.tile_pool(name="stage", bufs=6) as stage,
            tc.tile_pool(name="psum", bufs=4, space=bass.MemorySpace.PSUM) as psum,
        ):
            vt = vpool.tile([K, N], F32R)
            vtl = vpool.tile([K, HALF], F32R)
            nc.sync.dma_start(out=vt[:], in_=vt_d[:])
            nc.sync.dma_start(out=vtl[:], in_=vtl_d[:])

            cnt = 0
            for lvl in range(K):
                kl = lvl + 1  # contraction size at this level
                for i in range(RB):
                    ps = psum.tile([128, N], F32)
                    st = stage.tile([128, N], F32)
                    for j in range(2):
                        nc.tensor.matmul(
                            ps[:, j * 512:(j + 1) * 512],
                            lhsT=vtl[:kl, i * 128:(i + 1) * 128],
                            rhs=vt[:kl, j * 512:(j + 1) * 512],
                            start=True,
                            stop=True,
                        )
                    if cnt % 2 == 0:
                        nc.vector.tensor_copy(st[:], ps[:])
                    else:
                        nc.scalar.copy(st[:], ps[:])
                    nc.sync.dma_start(
                        out=out_d[lvl, i * 128:(i + 1) * 128, :], in_=st[:]
                    )
                    cnt += 1

    nc.compile()
    return nc


def _get_nc():
    global _nc_cache
    if _nc_cache is None:
        _nc_cache = _build()
    return _nc_cache


def _prepare_in_maps(evecs: np.ndarray) -> list[dict]:
    in_maps = []
    for c in range(NCORES):
        b, h = divmod(c, 2)
        vt = np.ascontiguousarray(evecs[b, 0].T, dtype=np.float32)  # [K, N]
        vtl = np.ascontiguousarray(vt[:, h * HALF:(h + 1) * HALF])
        in_maps.append({"vt": vt, "vtl": vtl})
    return in_maps


def _assemble(results: list[dict]) -> np.ndarray:
    out = np.empty((B, K, N, N), dtype=np.float32)
    for c in range(NCORES):
        b, h = divmod(c, 2)
        out[b, :, h * HALF:(h + 1) * HALF, :] = results[c]["out"]
    return out.reshape(B, K * C, N, N)


def kernel(evecs) -> np.ndarray:
    evecs = np.asarray(evecs, dtype=np.float32)
    assert evecs.shape == (B, C, N, K), evecs.shape
    nc = _get_nc()
    r = run_bass_kernel_spmd(nc, _prepare_in_maps(evecs), list(range(NCORES)))
    return _assemble(r.results)
